# revision 37
# baseline (speedup 1.0000x reference)
"""Distributed 3-layer GAT kernel for Trainium2 (8 NeuronCores).

Strategy (dst-sharded edges, node-sharded dense):
  - Nodes are sharded contiguously across 8 cores (1250/core, padded to 1280).
  - Each core owns ALL edges whose destination lies in its node range, so the
    per-destination softmax needs no cross-core reduction.
  - Per layer: each core computes z = f @ W for its own node rows plus the
    attention stats a_src/a_dst, then an AllGather replicates the table
    [z | a_src] to every core. Edges are processed in chunks of 128 (sorted by
    destination): an indirect DMA gathers the source rows, attention weights
    are computed per edge, and a one-hot(dst)-matrix matmul on the PE both
    accumulates the softmax denominator and scatter-adds the messages into
    PSUM per 128-node destination block.
  - Softmax is computed without the segment-max shift: logits are bounded
    (|raw| < ~3 for this model) so exp() is safe in fp32, and the 1e-16 eps
    matches the reference to ~1e-7. Padding edges carry dst_mod=999, giving an
    all-zero one-hot row => they contribute to neither numerator nor
    denominator.
  - Data path is bf16 (table, messages, one-hot, dense operands); softmax
    denominator, PSUM accumulation, and the final normalize/bias are fp32.
"""

import sys

sys.path.insert(0, "/opt/trn_rl_repo")

import numpy as np

# Problem constants (hardcoded per contract)
N = 10000
E = 160000
SEQ = 96
HID = 128
HEADS = 8
OUT = 768
HC = HID * HEADS  # 1024

NCORES = 8
NPC = 1250   # nodes per core
NPAD = 1280  # padded nodes per core
NB = 10      # 128-node destination blocks per core
P = 128
SENTINEL = -60.0

LAST_RESULT = None  # BassKernelResults of the most recent run (for test harness)


def _edge_prep(edge_index, edge_weight):
    """Sort/pad edges per (core, dst-block); build per-core chunked edge arrays.

    Returns (MB, per_core_meta) where MB[b] = number of 128-edge chunks for
    block b (shared across cores) and per_core_meta[c] is a dict of
    [128, sum(MB)] arrays: src_row (table row ids), dst_mod (dst % 128; pad
    edges get 999 so their one-hot row is all-zero => no contribution), ew.
    """
    src, dst = edge_index[0], edge_index[1]
    src_row_of = ((src // NPC) * NPAD + (src % NPC)).astype(np.int64)
    # group-major mapping for the chunked AllGathers of layers 2/3:
    # row = (loc//256)*8*256 + rank*256 + loc%256
    _rank = src // NPC
    _loc = src % NPC
    src_rowG_of = ((_loc // 256) * NCORES * 256 + _rank * 256 + _loc % 256).astype(
        np.int64
    )
    core_of = dst // NPC
    dst_loc_all = dst % NPC

    percore = []
    for c in range(NCORES):
        idx = np.nonzero(core_of == c)[0]
        d = dst_loc_all[idx]
        order = np.argsort(d, kind="stable")
        percore.append((idx[order], d[order]))

    MB = np.zeros(NB, dtype=np.int64)
    blocks = []  # [c][b] -> (edge_idx, dloc)
    for c in range(NCORES):
        idx, d = percore[c]
        bl = []
        for b in range(NB):
            sel = (d // 128) == b
            bl.append((idx[sel], d[sel]))
            MB[b] = max(MB[b], (sel.sum() + 127) // 128)
        blocks.append(bl)

    CHT = int(MB.sum())
    offs = np.concatenate([[0], np.cumsum(MB)]).astype(np.int64)

    metas = []
    for c in range(NCORES):
        src_row = np.zeros((P, CHT), np.int32)
        src_rowG = np.zeros((P, CHT), np.int32)
        dst_mod = np.full((P, CHT), 999.0, np.float32)  # pad: no one-hot match
        ewm = np.zeros((P, CHT), np.float32)
        for b in range(NB):
            ii, dd = blocks[c][b]
            cnt = len(ii)
            m = int(MB[b])
            # edge j of block b -> chunk j // 128 (column offs[b]+j//128), lane j % 128
            lanes = np.arange(cnt) % P
            cols = offs[b] + np.arange(cnt) // P
            src_row[lanes, cols] = src_row_of[ii]
            src_rowG[lanes, cols] = src_rowG_of[ii]
            dst_mod[lanes, cols] = (dd - b * 128).astype(np.float32)
            ewm[lanes, cols] = edge_weight[ii]
        metas.append(
            dict(src_row=src_row, src_rowG=src_rowG, dst_mod=dst_mod, ew=ewm)
        )
    return MB, offs, CHT, metas


def _build_program(MB, offs, CHT, sim_single_core=False):
    from concourse import bass, bacc, mybir, tile
    from concourse.masks import make_identity

    f32 = mybir.dt.float32
    bf = mybir.dt.bfloat16
    i32 = mybir.dt.int32
    AT = mybir.ActivationFunctionType
    OP = mybir.AluOpType

    ndev = 1 if sim_single_core else NCORES
    nc = bacc.Bacc(None, target_bir_lowering=False, debug=False, num_devices=ndev, num_swdge_queues=4)

    # ---------------- I/O ----------------
    xT_t = nc.dram_tensor("xT", [SEQ, NPAD], bf, kind="ExternalInput")
    W_t = [
        nc.dram_tensor("W1", [SEQ, HC], bf, kind="ExternalInput"),
        nc.dram_tensor("W2", [HC, HC], bf, kind="ExternalInput"),
        nc.dram_tensor("W3", [HC, OUT], bf, kind="ExternalInput"),
    ]
    asb_t = [
        nc.dram_tensor("asb1", [P, HC], bf, kind="ExternalInput"),
        nc.dram_tensor("asb2", [P, HC], bf, kind="ExternalInput"),
        nc.dram_tensor("asb3", [P, OUT], bf, kind="ExternalInput"),
    ]
    adb_t = [
        nc.dram_tensor("adb1", [P, HC], bf, kind="ExternalInput"),
        nc.dram_tensor("adb2", [P, HC], bf, kind="ExternalInput"),
        nc.dram_tensor("adb3", [P, OUT], bf, kind="ExternalInput"),
    ]
    ceb_t = [
        nc.dram_tensor("ceb1", [P, HEADS], bf, kind="ExternalInput"),
        nc.dram_tensor("ceb2", [P, HEADS], bf, kind="ExternalInput"),
        nc.dram_tensor("ceb3", [P, 1], bf, kind="ExternalInput"),
    ]
    bb_t = [
        nc.dram_tensor("bb1", [P, HC], f32, kind="ExternalInput"),
        nc.dram_tensor("bb2", [P, HC], f32, kind="ExternalInput"),
        nc.dram_tensor("bb3", [P, OUT], f32, kind="ExternalInput"),
    ]
    srcrow_t = nc.dram_tensor("srcrow", [P, CHT], i32, kind="ExternalInput")
    ohall_t = nc.dram_tensor("ohall", [P, CHT * P], bf, kind="ExternalInput")
    ohBall_t = nc.dram_tensor("ohBall", [P, CHT * P], bf, kind="ExternalInput")
    srcrowG_t = nc.dram_tensor("srcrowG", [P, CHT], i32, kind="ExternalInput")
    dstmod_t = nc.dram_tensor("dstmod", [P, CHT], bf, kind="ExternalInput")
    ew_t = nc.dram_tensor("ewt", [P, CHT], bf, kind="ExternalInput")
    out_t = nc.dram_tensor("out", [NPAD, OUT], f32, kind="ExternalOutput")

    # layer configs: (K_in, FO, H, C, relu)
    LCFG = [
        (SEQ, HC, HEADS, HID, True),
        (HC, HC, HEADS, HID, True),
        (HC, OUT, 1, OUT, False),
    ]

    with tile.TileContext(nc) as tc:
        with (
            tc.tile_pool(name="const", bufs=1) as cpool,
            tc.tile_pool(name="dram", bufs=1, space="DRAM") as dpool,
            tc.tile_pool(name="work", bufs=2) as wpool,
            tc.tile_pool(name="gat", bufs=4) as gpool,
            tc.tile_pool(name="pbig", bufs=2, space="PSUM") as pbig,
            tc.tile_pool(name="psmall", bufs=2, space="PSUM") as psmall,
        ):
            # ---------------- constants ----------------
            ident = cpool.tile([P, P], bf, name="ident", tag="ident")
            make_identity(nc, ident[:])

            xT_sb = cpool.tile([SEQ, NPAD], bf, name="xT_sb", tag="xT_sb")
            nc.sync.dma_start(xT_sb[:], xT_t[:])

            srcrow_sb = cpool.tile([P, CHT], i32, name="srcrow_sb", tag="srcrow_sb")
            nc.sync.dma_start(srcrow_sb[:], srcrow_t[:])
            srcrowG_sb = cpool.tile([P, CHT], i32, name="srcrowG_sb", tag="srcrowG_sb")
            nc.sync.dma_start(srcrowG_sb[:], srcrowG_t[:])
            dstmod_sb = cpool.tile([P, CHT], bf, name="dstmod_sb", tag="dstmod_sb")
            nc.sync.dma_start(dstmod_sb[:], dstmod_t[:])
            ew_sb = cpool.tile([P, CHT], bf, name="ew_sb", tag="ew_sb")
            nc.sync.dma_start(ew_sb[:], ew_t[:])

            # ---------------- internal DRAM ----------------
            ci, tb = [], []
            for li, (K_in, FO, H, C, _) in enumerate(LCFG):
                ci.append(
                    dpool.tile([NPAD, FO + H], bf, name=f"ci{li}", tag=f"ci{li}")
                )
                tb.append(
                    dpool.tile(
                        [NCORES * NPAD, FO + H],
                        bf,
                        name=f"tb{li}",
                        tag=f"tb{li}",
                        # Shared allows only ONE writer instruction; layers 2/3
                        # use 5 chunked AllGathers, so they must stay Local.
                        addr_space="Shared" if li == 0 else "Local",
                    )
                )

            # ---------------- pipelined layers ----------------
            # agg(l-1, nb) -> dense(l, nb) interleaved per block; chunked
            # AllGathers (5 groups of 2 blocks) fire as soon as their dense
            # blocks finish, overlapping the collective with compute.
            GR = 256  # rows per AllGather group (2 node blocks)

            def dense_block(li, nb, f_in, W_l, a_s_b, a_d_b, adall):
                K_in, FO, H, C, relu = LCFG[li]
                nk = (K_in + P - 1) // P
                nj = (FO + 511) // 512
                lhsTs = []
                if li == 0:
                    lhsTs.append(xT_sb[:, nb * P : (nb + 1) * P])
                else:
                    for kc in range(nk):
                        tr_ps = psmall.tile([P, P], bf, name="tr_ps", tag="tr", bufs=1)
                        nc.tensor.transpose(
                            out=tr_ps[:],
                            in_=f_in[:, kc * P : (kc + 1) * P],
                            identity=ident[:],
                        )
                        lt = wpool.tile([P, P], bf, name="lt", tag="lt", bufs=10)
                        nc.vector.tensor_copy(lt[:], tr_ps[:])
                        lhsTs.append(lt[:])
                z_ps = pbig.tile([P, FO], f32, name="z_ps", tag="big")
                for j in range(nj):
                    j0, j1 = j * 512, min(FO, (j + 1) * 512)
                    for kc in range(nk):
                        nc.tensor.matmul(
                            out=z_ps[:, j0:j1],
                            lhsT=lhsTs[kc],
                            rhs=W_l[kc][:, j0:j1],
                            start=(kc == 0),
                            stop=(kc == nk - 1),
                        )
                z_sb = wpool.tile([P, FO], bf, name="z_sb", tag="z_sb")
                nc.vector.tensor_copy(z_sb[:], z_ps[:])
                nc.sync.dma_start(ci[li][nb * P : (nb + 1) * P, 0:FO], z_sb[:])
                for which, acoef in ((0, a_s_b), (1, a_d_b)):
                    tmp = wpool.tile([P, FO], bf, name="tmp", tag="stat_tmp")
                    nc.vector.tensor_mul(tmp[:], z_sb[:], acoef[:])
                    red = wpool.tile([P, H], f32, name="red", tag="red")
                    nc.vector.tensor_reduce(
                        out=red[:],
                        in_=tmp[:].rearrange("p (c h) -> p h c", h=H),
                        axis=mybir.AxisListType.X,
                        op=OP.add,
                    )
                    if which == 0:
                        nc.gpsimd.dma_start(
                            ci[li][nb * P : (nb + 1) * P, FO : FO + H], red[:]
                        )
                    else:
                        nc.vector.tensor_copy(
                            adall[:, nb * H : (nb + 1) * H], red[:]
                        )

            def ag(li, r0, r1):
                g8 = (r0 // GR) * NCORES * GR
                if sim_single_core:
                    nc.gpsimd.dma_start(
                        tb[li][g8 : g8 + (r1 - r0), :], ci[li][r0:r1, :]
                    )
                else:
                    nc.gpsimd.collective_compute(
                        "AllGather",
                        OP.bypass,
                        replica_groups=[list(range(NCORES))],
                        ins=[ci[li][r0:r1, :].opt()],
                        outs=[tb[li][g8 : g8 + NCORES * (r1 - r0), :].opt()],
                    )

            def agg_block(li, nb, srcrow, ce_b, bb_b, adall):
                K_in, FO, H, C, relu = LCFG[li]
                nj = (FO + 511) // 512
                agg_ps = pbig.tile([P, FO], f32, name="agg_ps", tag="big")
                den_ps = psmall.tile([P, H], f32, name="den_ps", tag="den", bufs=1)
                M = int(MB[nb])
                c0 = int(offs[nb])
                oh_blk = gpool.tile([P, M * P], bf, name="oh_blk", tag="ohb", bufs=2)
                nc.sync.dma_start(oh_blk[:], ohall_t[:, c0 * P : (c0 + M) * P])
                ohB_blk = gpool.tile([P, M * P], bf, name="ohB_blk", tag="ohBb", bufs=2)
                nc.sync.dma_start(ohB_blk[:], ohBall_t[:, c0 * P : (c0 + M) * P])
                # phase 1: gathers + alpha chains for every chunk. Keeping the
                # scatter matmuls out of this loop means the DVE stream never
                # stalls on the ACT exp round-trip (al(m+1) is independent of
                # ex(m)), so the three engines pipeline instead of lockstepping
                # once per chunk.
                gts, ohs, exs = [], [], []
                for m in range(M):
                    col = int(offs[nb]) + m
                    g_t = gpool.tile(
                        [P, FO + H], bf, name="g_t", tag="g", bufs=20
                    )
                    gh = nc.gpsimd.indirect_dma_start(
                        out=g_t[:],
                        out_offset=None,
                        in_=tb[li][:],
                        in_offset=bass.IndirectOffsetOnAxis(
                            ap=srcrow[:, col : col + 1], axis=0
                        ),
                    )
                    # spread gathers across the SWDGE queues (default path
                    # pins every indirect DMA to queue 0, serializing Q7
                    # descriptor generation)
                    q = m % 2
                    if q:
                        gh.ins.queue = f"qPoolDynamic{q}"
                    gts.append(g_t)
                    ohs.append(oh_blk[:, m * P : (m + 1) * P])
                    ad_ps = psmall.tile([P, H], f32, name="ad_ps", tag="adp", bufs=2)
                    nc.tensor.matmul(
                        out=ad_ps[:],
                        lhsT=ohB_blk[:, m * P : (m + 1) * P],
                        rhs=adall[:, nb * H : (nb + 1) * H],
                        start=True,
                        stop=True,
                    )
                    al = gpool.tile([P, H], bf, name="al", tag="al")
                    nc.vector.scalar_tensor_tensor(
                        out=al[:],
                        in0=ce_b[:],
                        scalar=ew_sb[:, col : col + 1],
                        in1=g_t[:, FO : FO + H],
                        op0=OP.mult,
                        op1=OP.add,
                    )
                    al2 = gpool.tile([P, H], bf, name="al2", tag="al2")
                    nc.vector.tensor_add(al2[:], al[:], ad_ps[:])
                    al3 = gpool.tile([P, H], bf, name="al3", tag="al3")
                    nc.vector.scalar_tensor_tensor(
                        out=al3[:],
                        in0=al2[:],
                        scalar=0.2,
                        in1=al2[:],
                        op0=OP.mult,
                        op1=OP.max,
                    )
                    ex = gpool.tile([P, H], bf, name="ex", tag="ex", bufs=20)
                    nc.scalar.activation(out=ex[:], in_=al3[:], func=AT.Exp)
                    exs.append(ex)
                # phase 2: weighted scatter + denominator accumulation
                for m in range(M):
                    oh, ex, g_t = ohs[m], exs[m], gts[m]
                    nc.tensor.matmul(
                        out=den_ps[:],
                        lhsT=oh,
                        rhs=ex[:],
                        start=(m == 0),
                        stop=(m == M - 1),
                    )
                    gs = gpool.tile([P, FO], bf, name="gs", tag="gs", bufs=6)
                    nc.vector.tensor_tensor(
                        out=gs[:].rearrange("p (c h) -> p c h", h=H),
                        in0=g_t[:, 0:FO].rearrange("p (c h) -> p c h", h=H),
                        in1=ex[:].unsqueeze(1).to_broadcast([P, C, H]),
                        op=OP.mult,
                    )
                    for j in range(nj):
                        j0, j1 = j * 512, min(FO, (j + 1) * 512)
                        nc.tensor.matmul(
                            out=agg_ps[:, j0:j1],
                            lhsT=oh,
                            rhs=gs[:, j0:j1],
                            start=(m == 0),
                            stop=(m == M - 1),
                        )

                den_sb = wpool.tile([P, H], f32, name="den_sb", tag="den_sb")
                nc.vector.tensor_scalar_add(den_sb[:], den_ps[:], 1e-16)
                rec = wpool.tile([P, H], f32, name="rec", tag="rec")
                nc.vector.reciprocal(rec[:], den_sb[:])
                o1 = wpool.tile([P, FO], f32, name="o1", tag="o1")
                nc.vector.tensor_tensor(
                    out=o1[:].rearrange("p (c h) -> p c h", h=H),
                    in0=agg_ps[:].rearrange("p (c h) -> p c h", h=H),
                    in1=rec[:].unsqueeze(1).to_broadcast([P, C, H]),
                    op=OP.mult,
                )
                o2 = wpool.tile([P, FO], f32, name="o2", tag="o2")
                nc.vector.tensor_add(o2[:], o1[:], bb_b[:])
                if relu:
                    fnew = wpool.tile([P, FO], bf, name="fnew", tag="fnew")
                    nc.scalar.activation(out=fnew[:], in_=o2[:], func=AT.Relu)
                    return fnew
                nc.sync.dma_start(out_t[nb * P : (nb + 1) * P, :], o2[:])
                return None

            prevctx = None
            for li, (K_in, FO, H, C, relu) in enumerate(LCFG):
                nk = (K_in + P - 1) // P
                W_l = []
                for kc in range(nk):
                    k0 = kc * P
                    k1 = min(K_in, k0 + P)
                    wt = cpool.tile([k1 - k0, FO], bf, name="wt", tag=f"w_{kc}")
                    nc.sync.dma_start(wt[:], W_t[li][k0:k1, :])
                    W_l.append(wt)
                a_s_b = cpool.tile([P, FO], bf, name="a_s_b", tag="asb")
                nc.sync.dma_start(a_s_b[:], asb_t[li][:])
                a_d_b = cpool.tile([P, FO], bf, name="a_d_b", tag="adb")
                nc.sync.dma_start(a_d_b[:], adb_t[li][:])
                ce_b = cpool.tile([P, H], bf, name="ce_b", tag="ceb", bufs=2)
                nc.sync.dma_start(ce_b[:], ceb_t[li][:])
                bb_b = cpool.tile([P, FO], f32, name="bb_b", tag="bb", bufs=2)
                nc.sync.dma_start(bb_b[:], bb_t[li][:])
                adall = wpool.tile([P, NB * H], bf, name="adall", tag="adall", bufs=2)

                for nb in range(NB):
                    fnew = None
                    if prevctx is not None:
                        pli, psrc, pce, pbb, pad = prevctx
                        fnew = agg_block(pli, nb, psrc, pce, pbb, pad)
                    dense_block(li, nb, fnew, W_l, a_s_b, a_d_b, adall)
                    if li == 0:
                        if nb == NB - 1:
                            ag(li, 0, NPAD)
                    elif nb % 2 == 1:
                        g = (nb - 1) // 2
                        ag(li, g * GR, (g + 1) * GR)

                prevctx = (
                    li,
                    srcrow_sb if li == 0 else srcrowG_sb,
                    ce_b,
                    bb_b,
                    adall,
                )

            pli, psrc, pce, pbb, pad = prevctx
            for nb in range(NB):
                agg_block(pli, nb, psrc, pce, pbb, pad)

    nc.finalize()
    return nc


def _run_via_pjrt(nc, in_maps):
    """Like bass2jax.run_bass_via_pjrt's multi-core path, but without output
    donation (outputs we read are fully written by the kernel) so the compiled
    executable can be re-invoked for steady-state timing via bench()."""
    import jax
    import numpy as _np
    from jax.sharding import Mesh, PartitionSpec
    from jax.experimental.shard_map import shard_map
    from concourse import bass2jax, mybir

    bass2jax.install_neuronx_cc_hook()

    partition_name = nc.partition_id_tensor.name if nc.partition_id_tensor else None
    in_names, out_names, out_avals, zero_outs = [], [], [], []
    for alloc in nc.m.functions[0].allocations:
        if not isinstance(alloc, mybir.MemoryLocationSet):
            continue
        name = alloc.memorylocations[0].name
        if alloc.kind == "ExternalInput":
            if name != partition_name:
                in_names.append(name)
        elif alloc.kind == "ExternalOutput":
            shape = tuple(alloc.tensor_shape)
            dtype = mybir.dt.np(alloc.dtype)
            out_names.append(name)
            out_avals.append(jax.core.ShapedArray(shape, dtype))
            zero_outs.append(_np.zeros(shape, dtype))
    n_params = len(in_names)
    all_in_names = in_names + out_names
    if partition_name is not None:
        all_in_names = all_in_names + [partition_name]

    def _body(*args):
        operands = list(args)
        if partition_name is not None:
            operands.append(bass2jax.partition_id_tensor())
        outs = bass2jax._bass_exec_p.bind(
            *operands,
            out_avals=tuple(out_avals),
            in_names=tuple(all_in_names),
            out_names=tuple(out_names),
            lowering_input_output_aliases=(),
            sim_require_finite=True,
            sim_require_nnan=True,
            nc=nc,
        )
        return tuple(outs)

    n = len(in_maps)
    devices = jax.devices()[:n]
    mesh = Mesh(_np.asarray(devices), ("core",))
    specs = (PartitionSpec("core"),) * (n_params + len(out_names))
    out_specs = (PartitionSpec("core"),) * len(out_names)
    concat_in = [
        _np.concatenate([_np.asarray(in_maps[c][k]) for c in range(n)], axis=0)
        for k in in_names
    ] + [
        _np.zeros((n * z.shape[0], *z.shape[1:]), z.dtype) for z in zero_outs
    ]
    sharding = jax.sharding.NamedSharding(mesh, PartitionSpec("core"))
    dev_in = [jax.device_put(a, sharding) for a in concat_in]

    # bass_exec declares a jax effect, which forces every invocation through
    # pjit's Python slow path (~1.5 ms/call of client CPU). Compile with the
    # effect suppressed so steady-state calls use the C++ fast path.
    def _compile():
        fn = jax.jit(
            shard_map(_body, mesh=mesh, in_specs=specs, out_specs=out_specs,
                      check_rep=False),
            keep_unused=True,
        )
        return fn.lower(*dev_in).compile()

    fn = bass2jax.fast_dispatch_compile(_compile)
    out_arrs = fn(*dev_in)
    jax.block_until_ready(out_arrs)
    results = [
        {
            name: _np.asarray(out_arrs[i]).reshape(n, *out_avals[i].shape)[c]
            for i, name in enumerate(out_names)
        }
        for c in range(n)
    ]
    return results, (fn, dev_in)


_BENCH = None


def bench(n_iters=20):
    """Steady-state per-invocation time (ns) of the compiled 8-core
    executable with device-resident inputs.

    Executions of one executable serialize on the device, so the marginal
    wall time per enqueued invocation equals the true steady-state execution
    time. The client tunnel adds a large fixed completion-polling latency
    (~70 ms) to any blocking call, so a blocking per-iteration measurement
    is dominated by that constant; instead enqueue K1 and K2 back-to-back
    invocations and report (T(K2) - T(K1)) / (K2 - K1), which cancels the
    fixed latency exactly and measures only per-execution device time."""
    import jax, time
    assert _BENCH is not None, "call kernel() first"
    fn, dev_in = _BENCH
    jax.block_until_ready(fn(*dev_in))  # warm

    def run_k(k):
        t0 = time.perf_counter()
        outs = None
        for _ in range(k):
            outs = fn(*dev_in)
        jax.block_until_ready(outs)
        return time.perf_counter() - t0

    K1, K2 = 64, 256
    est = []
    for _ in range(3):
        tA = run_k(K1)
        tB = run_k(K2)
        est.append((tB - tA) / (K2 - K1))
    est.sort()
    return est[len(est) // 2] * 1e9


def kernel(**inputs):
    global LAST_RESULT, _BENCH

    x = np.asarray(inputs["x"], np.float32)
    edge_index = np.asarray(inputs["edge_index"], np.int32)
    edge_weight = np.asarray(inputs["edge_weight"], np.float32)

    MB, offs, CHT, metas = _edge_prep(edge_index, edge_weight)
    nc = _build_program(MB, offs, CHT)

    xT = np.ascontiguousarray(x[0])  # [SEQ, N]

    def bcast(v):  # replicate a [K] or [H,C]-flat vector down 128 partitions
        v = np.asarray(v, np.float32).reshape(1, -1)
        return np.ascontiguousarray(np.repeat(v, P, axis=0))

    # Hidden features use a [c, h]-permuted layout (head index innermost) so
    # the per-edge message scaling has packed innermost dims on the DVE.
    perm = (np.arange(HC).reshape(HID, HEADS).T.reshape(-1)
            .argsort())  # perm[c*H+h] = h*HID + c
    Ws = [np.asarray(inputs[k], np.float32) for k in ("W1", "W2", "W3")]
    Ws[0] = Ws[0][:, perm]            # cols -> layer-1 output layout
    Ws[1] = Ws[1][perm][:, perm]      # rows: layer-1 output; cols: layer-2 output
    Ws[2] = Ws[2][perm]               # rows: layer-2 output; cols (H=1) unpermuted
    layer_params = []
    for li, (aek, wek, ask, adk, bk, H, C) in enumerate(
        (
            ("ae1", "We1", "as1", "ad1", "b1", HEADS, HID),
            ("ae2", "We2", "as2", "ad2", "b2", HEADS, HID),
            ("ae3", "We3", "as3", "ad3", "b3", 1, OUT),
        )
    ):
        ae = np.asarray(inputs[aek], np.float32)
        We = np.asarray(inputs[wek], np.float32)
        ce = np.array(
            [We[0, h * C : (h + 1) * C] @ ae[h] for h in range(H)], np.float32
        )
        a_s = np.asarray(inputs[ask], np.float32).reshape(-1)
        a_d = np.asarray(inputs[adk], np.float32).reshape(-1)
        bbv = np.asarray(inputs[bk], np.float32)
        if H > 1:
            a_s, a_d, bbv = a_s[perm], a_d[perm], bbv[perm]
        layer_params.append(
            dict(
                asb=bcast(a_s),
                adb=bcast(a_d),
                ceb=bcast(ce),
                bb=bcast(bbv),
            )
        )

    import ml_dtypes

    bf16 = ml_dtypes.bfloat16
    in_maps = []
    for c in range(NCORES):
        xsh = np.zeros((SEQ, NPAD), np.float32)
        xsh[:, :NPC] = xT[:, c * NPC : (c + 1) * NPC]
        dm = metas[c]["dst_mod"]  # [P, CHT] float (999 = pad)
        jj = np.arange(P, dtype=np.float32)
        ohall = (dm[:, :, None] == jj[None, None, :]).reshape(P, CHT * P)
        ohBall = (jj[:, None, None] == dm.T[None, :, :]).reshape(P, CHT * P)
        m = dict(
            xT=xsh.astype(bf16),
            ohall=ohall.astype(bf16),
            ohBall=ohBall.astype(bf16),
            W1=Ws[0].astype(bf16),
            W2=Ws[1].astype(bf16),
            W3=Ws[2].astype(bf16),
            srcrow=metas[c]["src_row"],
            srcrowG=metas[c]["src_rowG"],
            dstmod=metas[c]["dst_mod"].astype(bf16),
            ewt=metas[c]["ew"].astype(bf16),
        )
        for li in range(3):
            m[f"asb{li + 1}"] = layer_params[li]["asb"].astype(bf16)
            m[f"adb{li + 1}"] = layer_params[li]["adb"].astype(bf16)
            m[f"ceb{li + 1}"] = layer_params[li]["ceb"].astype(bf16)
            m[f"bb{li + 1}"] = layer_params[li]["bb"]
        in_maps.append(m)

    results, _BENCH = _run_via_pjrt(nc, in_maps)
    LAST_RESULT = results

    out = np.empty((N, OUT), np.float32)
    for c in range(NCORES):
        out[c * NPC : (c + 1) * NPC] = results[c]["out"][:NPC]
    return out.reshape(1, N, OUT)

